# revision 16
# baseline (speedup 1.0000x reference)
"""KPConv regressor on 8 Trainium2 NeuronCores (Bass/Tile).

N=50000 points, NN=32 neighbors, K=15 kernel points, D_IN=64, D_OUT=1024,
B=16 graphs, head 1024->512->256->152 (host-side; ~0.03% of FLOPs).

Data-parallel over points: each core handles 6272 (6250 + pad) points.
Per core the device kernel:
  - dma_gather (biased int16 indices) fetches per-edge rows of a packed
    table [feats bf16 x64 | pos f32 x3] (256B rows) from HBM,
  - computes kernel-point correlations h = relu(1 - |rel - kp|/sigma) on
    DVE+ACT engines,
  - stage B: per 128-edge chunk, one [128,64] LDW (F) + 2 matmuls against
    block-diagonal h (even/odd kernel-point split) accumulate g^T in PSUM
    with (k-parity, d) on partitions,
  - stage C: 8 k-pair matmuls against prepacked weights -> x [128,1024],
    leaky-relu, segment-pool matmul accumulates pooled sums in PSUM.
Host sums the 8 partial pooled tensors and runs the tiny MLP head.
"""
import os
import sys
sys.path.insert(0, "/opt/trn_rl_repo")
import numpy as np

N = 50000
NN = 32
K = 15
D = 64
DOUT = 1024
B = 16
SIGMA = 0.3
NCORES = 8
PPC = 6272              # padded points per core (49 * 128)
EPC = PPC * NN          # 200704 edges per core
NBLK = 49               # blocks of 128 points
GPB = 4                 # gathers (1024 idx) per block
BIAS = 32768

_cache = {}


def _build_module(debug=False):
    import concourse.bacc as bacc
    import concourse.mybir as mybir
    from concourse.tile import TileContext

    fp32 = mybir.dt.float32
    bf16 = mybir.dt.bfloat16
    i16 = mybir.dt.int16
    Alu = mybir.AluOpType
    Act = mybir.ActivationFunctionType

    nc = bacc.Bacc(trn_type="TRN2", num_devices=NCORES, num_swdge_queues=4)
    t_table = nc.dram_tensor("table", [N, 64], fp32, kind="ExternalInput")
    t_idx = nc.dram_tensor("idx", [128, EPC // 16], i16, kind="ExternalInput")
    t_psl = nc.dram_tensor("possl", [128, NBLK * 96], fp32, kind="ExternalInput")
    t_seg = nc.dram_tensor("seg", [128, NBLK * 16], bf16, kind="ExternalInput")
    t_wc = nc.dram_tensor("wc", [128, 8 * DOUT], bf16, kind="ExternalInput")
    t_kpc = nc.dram_tensor("kpc", [128, 48], fp32, kind="ExternalInput")
    t_kpn = nc.dram_tensor("kpn", [128, 16], fp32, kind="ExternalInput")
    t_out = nc.dram_tensor("pooled", [16, DOUT], fp32, kind="ExternalOutput")
    if debug:
        t_dhh = nc.dram_tensor("dhh", [128, 512], fp32, kind="ExternalOutput")
        t_drel = nc.dram_tensor("drel", [128, 96], fp32, kind="ExternalOutput")
        t_dgp = nc.dram_tensor("dgp", [128, 96], fp32, kind="ExternalOutput")
        t_dps = nc.dram_tensor("dps", [128, 96], fp32, kind="ExternalOutput")
        t_dd2 = nc.dram_tensor("dd2", [128, 512], fp32, kind="ExternalOutput")
        t_dg = nc.dram_tensor("dg", [128, 1024], fp32, kind="ExternalOutput")
        t_dy = nc.dram_tensor("dy", [128, 1024], fp32, kind="ExternalOutput")

    with TileContext(nc) as tc:
        with (
            tc.tile_pool(name="cst", bufs=1) as cst,
            tc.tile_pool(name="gat", bufs=3) as gat,
            tc.tile_pool(name="sta", bufs=2) as sta,
            tc.tile_pool(name="hbd", bufs=1) as hbp,
            tc.tile_pool(name="gsb", bufs=2) as gsp,
            tc.tile_pool(name="psg", bufs=2, space="PSUM") as psg,
            tc.tile_pool(name="psx", bufs=1, space="PSUM") as psx,
            tc.tile_pool(name="psp", bufs=1, space="PSUM") as psp,
        ):
            idx_sb = cst.tile([128, EPC // 16], i16)
            nc.sync.dma_start(out=idx_sb[:], in_=t_idx[:, :])
            psl = cst.tile([128, NBLK * 96], fp32)
            nc.sync.dma_start(out=psl[:], in_=t_psl[:, :])
            seg = cst.tile([128, NBLK, 16], bf16)
            nc.sync.dma_start(out=seg[:, :, :],
                              in_=t_seg[:, :].rearrange("p (b s) -> p b s", s=16))
            wc = cst.tile([128, 8, DOUT], bf16)
            nc.sync.dma_start(out=wc[:, :, :],
                              in_=t_wc[:, :].rearrange("p (k o) -> p k o", o=DOUT))
            kpc = cst.tile([128, 3, 16], fp32)
            nc.sync.dma_start(out=kpc[:, :, :],
                              in_=t_kpc[:, :].rearrange("p (c k) -> p c k", k=16))
            kpn = cst.tile([128, 16], fp32)
            nc.sync.dma_start(out=kpn[:], in_=t_kpn[:, :])

            # persistent double-buffered block-diagonal h tiles (even/odd k)
            hbd = [[hbp.tile([128, 32, 32], bf16, tag=f"hbd{s}{par}",
                             name=f"hbd{s}{par}")
                    for par in range(2)] for s in range(2)]
            for s in range(2):
                for par in range(2):
                    nc.vector.memset(hbd[s][par][:, :, :], 0.0)

            pooled_ps = psp.tile([16, DOUT], fp32, space="PSUM")

            for b in range(NBLK):
                gt = gat.tile([128, 32, 64], fp32, tag="gt")
                for gi in range(GPB):
                    g = b * GPB + gi
                    nc.gpsimd.dma_gather(
                        out_ap=gt[:, gi * 8:(gi + 1) * 8, :],
                        in_ap=t_table[BIAS:, :],
                        idxs_ap=idx_sb[:, g * 64:(g + 1) * 64],
                        num_idxs=1024,
                        num_idxs_reg=1024,
                        elem_size=64,
                        queue_num=gi,
                    )
                # rel = gathered pos - self pos          [128, 32, 3]
                rel = sta.tile([128, 32, 3], fp32, tag="rel")
                nc.vector.tensor_tensor(
                    out=rel[:, :, :], in0=gt[:, :, 32:35],
                    in1=psl[:, b * 96:(b + 1) * 96].rearrange(
                        "p (c x) -> p c x", x=3),
                    op=Alu.subtract)
                sq = sta.tile([128, 32, 3], fp32, tag="sq")
                nc.vector.tensor_tensor(out=sq[:, :, :], in0=rel[:, :, :],
                                        in1=rel[:, :, :], op=Alu.mult)
                rn = sta.tile([128, 32], fp32, tag="rn")
                nc.vector.tensor_reduce(out=rn[:, :], in_=sq[:, :, :],
                                        axis=mybir.AxisListType.X, op=Alu.add)
                # d2[e, kk] = rn[e] + kpn[kk] + sum_c rel_c * kpc[c, kk]
                d2 = sta.tile([128, 32, 16], fp32, tag="d2")
                nc.vector.scalar_tensor_tensor(
                    out=d2[:, :, :],
                    in0=rn[:, :].unsqueeze(2).to_broadcast([128, 32, 16]),
                    scalar=1.0,
                    in1=kpn[:, :].unsqueeze(1).to_broadcast([128, 32, 16]),
                    op0=Alu.mult, op1=Alu.add)
                tmp = sta.tile([128, 32, 16], fp32, tag="tmp")
                for c in range(3):
                    nc.vector.tensor_tensor(
                        out=tmp[:, :, :],
                        in0=rel[:, :, c:c + 1].to_broadcast([128, 32, 16]),
                        in1=kpc[:, c, :].unsqueeze(1).to_broadcast([128, 32, 16]),
                        op=Alu.mult)
                    nc.vector.tensor_tensor(out=d2[:, :, :], in0=d2[:, :, :],
                                            in1=tmp[:, :, :], op=Alu.add)
                nc.vector.tensor_scalar_max(out=d2[:, :, :], in0=d2[:, :, :],
                                            scalar1=0.0)
                dd = sta.tile([128, 32, 16], fp32, tag="dd")
                nc.scalar.activation(out=dd[:, :, :], in_=d2[:, :, :], func=Act.Sqrt)
                hh = sta.tile([128, 32, 16], bf16, tag="hh")
                nc.scalar.activation(out=hh[:, :, :], in_=dd[:, :, :], func=Act.Relu,
                                     bias=1.0, scale=-1.0 / SIGMA)
                # scatter h into block-diagonal tiles (4 partition groups x 2 par)
                s = b % 2
                for par in range(2):
                    for i in range(4):
                        nc.sync.dma_start(
                            out=hbd[s][par][32 * i:32 * (i + 1), :,
                                            8 * i:8 * (i + 1)],
                            in_=hh[32 * i:32 * (i + 1), :,
                                   8 * par:8 * (par + 1)])
                # stage B: per chunk LDW F + 2 matmuls -> G^T in PSUM
                psumG = psg.tile([128, 1024], fp32, space="PSUM", tag="psumG")
                for c in range(32):
                    fap = gt[:, c, 0:32].bitcast(bf16)
                    nc.tensor.matmul(
                        out=psumG[0:64, c * 32:(c + 1) * 32],
                        lhsT=fap, rhs=hbd[s][0][:, c, :],
                        start=True, stop=True)
                    nc.tensor.matmul(
                        out=psumG[64:128, c * 32:(c + 1) * 32],
                        lhsT=fap, rhs=hbd[s][1][:, c, :],
                        start=True, stop=True)
                # reorder (c, pt, kk) -> (kk, c, pt), cast bf16
                gsb = gsp.tile([128, 8, 128], bf16, tag="gsb")
                nc.vector.tensor_copy(
                    out=gsb[:, :, :],
                    in_=psumG[:, :].rearrange("p (c pt k) -> p k (c pt)",
                                              c=32, pt=4, k=8))
                # stage C: 8 k-pair matmuls, accumulate x in PSUM
                psumX = psx.tile([128, 1024], fp32, space="PSUM", tag="psumX")
                for kk in range(8):
                    for hf in range(2):
                        nc.tensor.matmul(
                            out=psumX[:, hf * 512:(hf + 1) * 512],
                            lhsT=gsb[:, kk, :],
                            rhs=wc[:, kk, hf * 512:(hf + 1) * 512],
                            start=(kk == 0), stop=(kk == 7))
                # leaky relu -> bf16
                y = gsp.tile([128, 1024], bf16, tag="y")
                xs = gsp.tile([128, 1024], bf16, tag="xs")
                nc.vector.tensor_scalar_mul(out=xs[:, :], in0=psumX[:, :],
                                            scalar1=0.1)
                nc.vector.tensor_tensor(out=y[:, :], in0=psumX[:, :],
                                        in1=xs[:, :], op=Alu.max)
                if debug and b == 0:
                    dbg6 = gsp.tile([128, 96], fp32, name="dbg6")
                    nc.vector.tensor_copy(out=dbg6[:, :].rearrange("p (c x) -> p c x", x=3),
                                          in_=gt[:, :, 32:35])
                    nc.sync.dma_start(out=t_dgp[:, :], in_=dbg6[:, :])
                    dbg7 = gsp.tile([128, 96], fp32, name="dbg7")
                    nc.vector.tensor_copy(out=dbg7[:, :], in_=psl[:, 0:96])
                    nc.sync.dma_start(out=t_dps[:, :], in_=dbg7[:, :])
                    dbg4 = gsp.tile([128, 96], fp32, name="dbg4")
                    nc.vector.tensor_copy(out=dbg4[:, :],
                                          in_=rel[:, :, :].rearrange("p c x -> p (c x)"))
                    nc.sync.dma_start(out=t_drel[:, :], in_=dbg4[:, :])
                    dbg5 = gsp.tile([128, 512], fp32, name="dbg5")
                    nc.vector.tensor_copy(out=dbg5[:, :],
                                          in_=d2[:, :, :].rearrange("p c k -> p (c k)"))
                    nc.sync.dma_start(out=t_dd2[:, :], in_=dbg5[:, :])
                    dbg1 = gsp.tile([128, 512], fp32, name="dbg1")
                    nc.vector.tensor_copy(out=dbg1[:, :],
                                          in_=hh[:, :, :].rearrange("p c k -> p (c k)"))
                    nc.sync.dma_start(out=t_dhh[:, :], in_=dbg1[:, :])
                    dbg2 = gsp.tile([128, 1024], fp32, name="dbg2")
                    nc.vector.tensor_copy(out=dbg2[:, :],
                                          in_=gsb[:, :, :].rearrange("p k n -> p (k n)"))
                    nc.sync.dma_start(out=t_dg[:, :], in_=dbg2[:, :])
                    dbg3 = gsp.tile([128, 1024], fp32, name="dbg3")
                    nc.vector.tensor_copy(out=dbg3[:, :], in_=y[:, :])
                    nc.sync.dma_start(out=t_dy[:, :], in_=dbg3[:, :])
                # segment pooling (accumulated across blocks)
                for hf in range(2):
                    nc.tensor.matmul(
                        out=pooled_ps[:, hf * 512:(hf + 1) * 512],
                        lhsT=seg[:, b, :], rhs=y[:, hf * 512:(hf + 1) * 512],
                        start=(b == 0), stop=(b == NBLK - 1))

            pooled_sb = cst.tile([16, DOUT], fp32)
            nc.vector.tensor_copy(out=pooled_sb[:, :], in_=pooled_ps[:, :])
            nc.sync.dma_start(out=t_out[:, :], in_=pooled_sb[:, :])

    nc.compile()
    return nc


def _prep_inputs(pos, feats, kernel_points, kp_weights, neighbor_idx, batch):
    import ml_dtypes

    # packed gather table: per row 64 bf16 feats (128B) + 3 f32 pos + pad
    raw = np.zeros((N, 256), np.uint8)
    raw[:, :128] = feats.astype(ml_dtypes.bfloat16).view(np.uint8)
    raw[:, 128:140] = pos.astype(np.float32).view(np.uint8)
    table = raw.view(np.float32)

    # prepacked stage-C weights: pair kk -> [W[2kk]; W[2kk+1]] stacked on rows
    wcm = np.zeros((128, 8 * DOUT), ml_dtypes.bfloat16)
    for kk in range(8):
        wcm[0:64, kk * DOUT:(kk + 1) * DOUT] = kp_weights[2 * kk].astype(
            ml_dtypes.bfloat16)
        if 2 * kk + 1 < K:
            wcm[64:128, kk * DOUT:(kk + 1) * DOUT] = kp_weights[2 * kk + 1].astype(
                ml_dtypes.bfloat16)

    # kernel-point constants, parity-interleaved column order:
    # col kk in [0,8) -> k = 2*kk (even), col kk in [8,16) -> k = 2*(kk-8)+1
    korder = [2 * i for i in range(8)] + [2 * i + 1 for i in range(8)]
    kpcm = np.zeros((128, 48), np.float32)
    kpnm = np.zeros((128, 16), np.float32)
    for col, k in enumerate(korder):
        if k < K:
            kpcm[:, 0 * 16 + col] = -2.0 * kernel_points[k, 0]
            kpcm[:, 1 * 16 + col] = -2.0 * kernel_points[k, 1]
            kpcm[:, 2 * 16 + col] = -2.0 * kernel_points[k, 2]
            kpnm[:, col] = (kernel_points[k] ** 2).sum()
        else:
            kpnm[:, col] = 1e9

    per_core = []
    for c in range(NCORES):
        lo = c * (N // NCORES)
        hi = lo + (N // NCORES)
        npts = hi - lo
        nbr = np.zeros((PPC, NN), np.int64)
        nbr[:npts] = neighbor_idx[lo:hi]
        nbr[npts:] = BIAS  # pad points: gather a real row, seg zero kills them
        pp = np.zeros((PPC, 3), np.float32)
        pp[:npts] = pos[lo:hi]
        bb = np.full((PPC,), -1, np.int64)
        bb[:npts] = batch[lo:hi]

        # tail-trim guard: each 32-point gather group must end with an edge
        # whose source index >= BIAS (biased int16 stays >= 0)
        for g in range(PPC // 32):
            blk = slice(g * 32, (g + 1) * 32)
            nb = nbr[blk]
            has_hi = (nb >= BIAS).any(axis=1)
            if not has_hi[31]:
                j = int(np.argmax(has_hi))
                assert has_hi[j], "no high neighbor in gather group"
                for arr in (nbr, pp, bb):
                    t = arr[g * 32 + 31].copy()
                    arr[g * 32 + 31] = arr[g * 32 + j]
                    arr[g * 32 + j] = t
            row = nbr[g * 32 + 31]
            jhi = int(np.argmax(row >= BIAS))
            row[31], row[jhi] = row[jhi], row[31]

        idx16 = (nbr.reshape(-1) - BIAS).astype(np.int16)       # [EPC]
        idxm = np.zeros((128, EPC // 16), np.int16)
        blocks = idx16.reshape(EPC // 1024, 64, 16)             # [196, 64, 16]
        wrapped = np.transpose(blocks, (2, 0, 1)).reshape(16, EPC // 16)
        for grp in range(8):
            idxm[grp * 16:(grp + 1) * 16, :] = wrapped
        # edge-major self positions: edge e -> [e%128, e//128 within block]
        ppe = np.repeat(pp, NN, axis=0)                         # [EPC, 3]
        pslm = np.ascontiguousarray(
            ppe.reshape(NBLK, 32, 128, 3).transpose(2, 0, 1, 3)).reshape(
            128, NBLK * 96)
        segm = np.zeros((PPC, 16), np.float32)
        valid = bb >= 0
        segm[np.arange(PPC)[valid], bb[valid]] = 1.0
        segm = np.ascontiguousarray(
            segm.reshape(NBLK, 128, 16).transpose(1, 0, 2)).reshape(
            128, NBLK * 16).astype(ml_dtypes.bfloat16)
        per_core.append({
            "table": table, "idx": idxm, "possl": pslm, "seg": segm,
            "wc": wcm, "kpc": kpcm, "kpn": kpnm,
        })
    return per_core


def kernel(pos, feats, kernel_points, kp_weights, w1, b1, w2, b2, w3, b3,
           neighbor_idx, batch):
    from concourse.bass_utils import run_bass_kernel_spmd

    pos = np.asarray(pos, np.float32)
    feats = np.asarray(feats, np.float32)
    kernel_points = np.asarray(kernel_points, np.float32)
    kp_weights = np.asarray(kp_weights, np.float32)
    neighbor_idx = np.asarray(neighbor_idx)
    batch = np.asarray(batch)

    if "nc" not in _cache:
        _cache["nc"] = _build_module()
    nc = _cache["nc"]

    in_maps = _prep_inputs(pos, feats, kernel_points, kp_weights,
                           neighbor_idx, batch)
    res = run_bass_kernel_spmd(nc, in_maps, core_ids=list(range(NCORES)),
                               trace=bool(os.environ.get("KTRACE")))
    _cache["last_res"] = res

    pooled_sum = np.zeros((B, DOUT), np.float64)
    for r in res.results:
        pooled_sum += r["pooled"].astype(np.float64)
    counts = np.bincount(batch, minlength=B).astype(np.float64)
    pooled = (pooled_sum / np.maximum(counts, 1.0)[:, None]).astype(np.float32)

    h1 = np.maximum(pooled @ np.asarray(w1, np.float32) + np.asarray(b1, np.float32), 0)
    h2 = np.maximum(h1 @ np.asarray(w2, np.float32) + np.asarray(b2, np.float32), 0)
    out = h2 @ np.asarray(w3, np.float32) + np.asarray(b3, np.float32)
    return np.asarray(out, np.float32)
